# revision 59
# baseline (speedup 1.0000x reference)
"""Trainium2 Bass kernel for nn_DeepAttensionModule (cumulative set attention).

Self-contained: takes the FULL unsharded inputs of reference.setup_inputs(),
returns the FULL [4, 2048, 64] float32 output.

Strategy (v2)
-------------
Data-parallel over batch B=4: one NeuronCore per batch element (cores 0-3).
Per core, everything is channel-major [C, P=2048]; matmuls weight-stationary
fp32r.

Measured (wall-slope over a HW repeat loop, incl. per-iteration const
loads): ~40us/iter vs the 81.5us v1 harness baseline.

v2 changes vs v1 (81us baseline):
- ALL folded weights ship in ONE packed f32r const DMA + one f32 scalar DMA
  (v1 issued ~17 serialized 625ns DMAs on one queue).
- comb is built PAIR-PACKED [63, 1024]: partition group 0 holds even segs,
  group 1 odd segs; the one-hot is_equal writes straight into it.
- sin on DVE with a factored degree-7 polynomial after round-to-nearest
  range reduction, and 1/den via the single-instruction DVE
  reciprocal_approx_fast instead of ACT exp(-ln(den)) -> the whole kernel
  uses one activation table set (v1 thrashed ~10 x 1.3us table loads).
- everything in bf16 except the scans/reciprocal/carries (fp32) -- rel err
  6.9e-3 vs the 2e-2 gate.
- 32/64-row stages pack 2-4 segments across the 128 partitions via
  block-diagonal zero-padded lhsT (dst partition base always 0 -- PE column
  tiling is broken in this toolchain): psi MLP, the cumulative-psi scan, agg
  scale, and rho-2 each run as one or two [128,512] ops instead of 4 narrow
  ones. Cross-seg scan carries are applied for free in consumers (activation
  bias / scalar_tensor_tensor scalar); the agg carry uses a tiny 0/1-matrix
  matmul on PE.
- num/den scans chain across segs (init = previous last column) so they
  are globally cumulative: no carry application at all, and the normalize
  becomes a plain multiply (split DVE/Pool). X-multiply runs on Pool.
- every PSUM tag is double-buffered in exactly 8 banks via tag sharing
  (carry-matmul rides the psi1 tag, rho-2 rides the phi2 tag), which
  un-serializes the rho-MLP tail.
"""
import numpy as np

import concourse.bacc as bacc
import concourse.mybir as mybir
import concourse.tile as tile
from concourse import bass_utils

B, P = 4, 2048
NUM_MODS, D_TIME = 22, 8
DIM_S = NUM_MODS + D_TIME + 1          # 31
PHI_W, PSI_W, PSI_LAT = 32, 32, 32
DOT, HEADS, RHO_W = 16, 4, 64
N_CORES = 4
SEGW = P // 4                           # 512
NCH = P // 128                          # 16 seq chunks in the [128,128] reshape

F32 = mybir.dt.float32
F32R = mybir.dt.float32r
BF16 = mybir.dt.bfloat16
I32 = mybir.dt.int32
AF = mybir.ActivationFunctionType
OP = mybir.AluOpType

# factored degree-7 odd minimax: sin(2*pi*d) ~= d*c7*(u-r1)*(u*u+pp*u+qq),
# u=d^2, |d|<=0.5, abs err 6.7e-4
SIN_C7 = -57.11540449516585
SIN_R1 = 0.24989525578673413
SIN_P = -1.1214834306739268
SIN_Q = 0.4399767340537331

# bf16 weights block columns (shipped as bf16 from the host -- no casting
# DMA, no f32r rounding constraints; PE speed is identical).
# ALL matmuls use lhsT/rhs/dst partition base 0: group selection is embedded
# as zero-padding in the weights (non-base-0 stationary loads hang the PE).
C_W1X2 = 0          # [64, 128] blockdiag(w1p, w1p); pad rows 31,63 zero
C_W2PL = 128        # [128, 128] psi2 blocks for h1 pair 0 -> pspsi rows 0..63
C_W2PH = 256        # [128, 128] psi2 blocks for h1 pair 1 -> rows 64..127
C_PHI2 = 384        # [128, 128] x2 parities: phi2 4x-rep, other half zero
C_WSA2 = 640        # [64, 128] x2 parities: folded logit weights (comb part)
C_WSF4 = 896        # [128, 128] x4 segs: folded logit weights (agg part)
C_R1 = 1408         # [128, 64]
C_R2B = 1472        # [128, 128] blockdiag(rho_w2, rho_w2)
C_T128 = 1600       # [128, 128] 0/1 exclusive-prefix-group matrix
CW_END = 1728
# f32 scalar block columns
S_B1X2 = 0          # [128, 1] psi_b1|phi_b1 twice
S_BPSI4 = 1         # [128, 1] psi_b2 4x
S_BPHI4 = 2         # [128, 1] phi_b2 4x
S_RB1X2 = 3         # [128, 1]
S_RB2X2 = 4         # [128, 1]
S_SC = 5            # [128, 1] tenc scale per reshape row
S_SH = 6            # [128, 1] tenc shift per reshape row
S_IO = 7            # iota 1..22 in partition groups 0 and 32
S_ZERO = 8          # all-zero column (pad-row fill source)
S_RC = 9            # [128, SEGW] 1/(512*s+c+1) in partition group s
NS = 9 + SEGW
NCW = CW_END + NS


def build(repeat: int = 1, depth: int = 99):
    nc = bacc.Bacc("TRN2", target_bir_lowering=False, debug=False,
                   num_devices=N_CORES)

    times = nc.dram_tensor("times", [1, P], F32, kind="ExternalInput").ap()
    values = nc.dram_tensor("values", [1, P], F32, kind="ExternalInput").ap()
    meas = nc.dram_tensor("meas", [1, P], F32, kind="ExternalInput").ap()
    cpw = nc.dram_tensor("cpw", [128, CW_END], BF16,
                         kind="ExternalInput").ap()
    cpsd = nc.dram_tensor("cps", [128, NS], F32,
                          kind="ExternalInput").ap()
    out = nc.dram_tensor("out", [RHO_W, P], F32, kind="ExternalOutput").ap()

    segs = [slice(s * SEGW, (s + 1) * SEGW) for s in range(4)]
    prs = [slice(p * SEGW, (p + 1) * SEGW) for p in range(2)]  # pair cols

    with tile.TileContext(nc) as tc:
        with tc.tile_pool(name="const", bufs=1) as cpool, \
             tc.tile_pool(name="work", bufs=1) as pool, \
             tc.tile_pool(name="psum", bufs=1, space="PSUM") as pp, \
             tc.tile_pool(name="psum2", bufs=2, space="PSUM") as pp2, \
             tc.tile_pool(name="dram", bufs=1, space="DRAM") as dp:

            CR = cpool.tile([128, CW_END], BF16, tag="cpackw")
            CP = cpool.tile([128, NS], F32, tag="cpacks")
            state = {"first": True}

            def body():
                # ---------------- comb assembly (pair-packed) --------------
                # row group layout (x2 at base 0/32 for even/odd seg of the
                # pair): [pad, onehot*22, values, tenc*8] = 32 rows.
                # columns: pair 0 = segs 0,1 | pair 1 = segs 2,3
                comb = pool.tile([64, P // 2], BF16, tag="comb")

                # tenc source load first (gates the longest front-end chain)
                t128 = pool.tile([128, 128], F32, tag="t128")
                nc.sync.dma_start(
                    t128[:, :],
                    times.rearrange("o (k i) -> o k i", i=128).broadcast_to(
                        [8, NCH, 128]))
                # values rows 23 / 55 via casting SWDGE DMAs straight from
                # DRAM -- idle Pool queue, ready long before comb is needed
                vview = values.rearrange("o (p j c) -> o j p c", p=2, j=2)
                for jj in range(2):
                    nc.gpsimd.dma_start(comb[32 * jj + 23:32 * jj + 24, :],
                                        vview[:, jj])
                if state["first"]:
                    # hot scalar columns first (they gate the tenc chain);
                    # the RC block and the weight pack can land later
                    nc.scalar.dma_start(CP[:, 0:8], cpsd[:, 0:8])
                    state["first"] = False

                mb = pool.tile([32, P], F32, tag="mb")
                nc.scalar.dma_start(mb[:, :], meas.broadcast_to([32, P]))


                # tenc rows 23..30 / 55..62
                q = pool.tile([128, 128], F32, tag="q")
                nc.vector.tensor_scalar(q[:, :], t128[:, :],
                                        CP[:, S_SC:S_SC + 1],
                                        CP[:, S_SH:S_SH + 1],
                                        OP.mult, OP.add)
                ni = pool.tile([128, 128], I32, tag="ni")
                nc.vector.tensor_copy(ni[:, :], q[:, :])
                nf = pool.tile([128, 128], F32, tag="nf")
                nc.vector.tensor_copy(nf[:, :], ni[:, :])
                d8 = pool.tile([128, 128], F32, tag="d8")
                nc.vector.scalar_tensor_tensor(
                    out=d8[:, :], in0=nf[:, :], scalar=-1.0, in1=q[:, :],
                    op0=OP.mult, op1=OP.add)
                # sin(2*pi*d) = (((u-r1)*d) * ((u+pp)*u+qq)) * c7
                uu = pool.tile([128, 128], F32, tag="uu")
                nc.vector.tensor_tensor(uu[:, :], d8[:, :], d8[:, :], OP.mult)
                t1 = pool.tile([128, 128], F32, tag="t1")
                nc.vector.scalar_tensor_tensor(
                    out=t1[:, :], in0=uu[:, :], scalar=-SIN_R1, in1=d8[:, :],
                    op0=OP.add, op1=OP.mult)
                vv = pool.tile([128, 128], F32, tag="vv")
                nc.vector.scalar_tensor_tensor(
                    out=vv[:, :], in0=uu[:, :], scalar=SIN_P, in1=uu[:, :],
                    op0=OP.add, op1=OP.mult)
                ww = pool.tile([128, 128], F32, tag="ww")
                nc.vector.scalar_tensor_tensor(
                    out=ww[:, :], in0=vv[:, :], scalar=SIN_Q, in1=t1[:, :],
                    op0=OP.add, op1=OP.mult)
                tsin = pool.tile([128, 128], BF16, tag="tsin")
                nc.vector.tensor_scalar(tsin[:, :], ww[:, :], SIN_C7, None,
                                        OP.mult)
                # relayout via f32r DRAM bounce (SBUF-src DMAs cannot walk
                # multi-level partition patterns; DRAM-src can). tdram
                # partition index is j*16+p*8+jj*4+k (j=tenc row, p=pair,
                # jj=parity, k=chunk), payload col i -> comb col 512p+128k+i
                tdram = dp.tile([128, 128], BF16, tag="tdram")
                nc.scalar.dma_start(tdram[:, :], tsin[:, :])
                tgat = tdram[:, :].rearrange("(j p jj k) i -> jj j p k i",
                                             j=8, p=2, jj=2, k=4)
                nc.sync.dma_start(comb[24:32, :], tgat[0])
                nc.scalar.dma_start(comb[56:64, :], tgat[1])

                # pad + one-hot rows g+0..g+22 (emitted after the tenc chain
                # so the DVE runs the latency-critical sin path first); io row
                # g+0 is -1 so the compare zero-fills the pad row. No write
                # overlaps values/tenc rows -> order-free.
                for s in range(4):
                    g, pr = 32 * (s % 2), prs[s // 2]
                    nc.vector.tensor_scalar(
                        comb[g:g + 23, pr], mb[0:23, segs[s]],
                        CP[g:g + 23, S_IO:S_IO + 1], None, OP.is_equal)

                if state.get("cr", True):
                    nc.sync.dma_start(CR[:, :], cpw)
                    nc.scalar.dma_start(CP[:, 8:NS], cpsd[:, 8:NS])
                    state["cr"] = False

                if depth <= 1:
                    nc.gpsimd.dma_start(out[:, 0:P // 2], comb[:, :])
                    return

                # ---------------- phase A: psi branch (packed) -------------
                h1 = pool.tile([128, P // 2], BF16, tag="h1")  # pair-packed

                ps1 = [pp2.tile([128, SEGW], F32, tag="ps1",
                                name=f"ps1_{p}") for p in range(2)]
                for p in range(2):
                    nc.tensor.matmul(ps1[p][:, :],
                                     CR[0:64, C_W1X2:C_W1X2 + 128],
                                     comb[0:64, prs[p]],
                                     start=True, stop=True)
                    nc.scalar.activation(h1[:, prs[p]], ps1[p][:, :],
                                         AF.Relu,
                                         bias=CP[:, S_B1X2:S_B1X2 + 1])
                if depth <= 2:
                    nc.sync.dma_start(out[:, 0:P // 2],
                                      h1[0:64, :].bitcast(F32))
                    return

                # psi2: 4x32 partition-packed via two accumulating matmuls
                pspsi = pp2.tile([128, SEGW], F32, tag="mm128",
                                 name="pspsi")
                nc.tensor.matmul(pspsi[:, :], CR[:, C_W2PL:C_W2PL + 128],
                                 h1[:, prs[0]], start=True, stop=False)
                nc.tensor.matmul(pspsi[:, :], CR[:, C_W2PH:C_W2PH + 128],
                                 h1[:, prs[1]], start=False, stop=True)
                encpsi = pool.tile([128, SEGW], F32, tag="encpsi")
                totpsi = pool.tile([128, 1], F32, tag="totpsi")
                nc.scalar.activation(encpsi[:, :], pspsi[:, :], AF.Relu,
                                     bias=CP[:, S_BPSI4:S_BPSI4 + 1],
                                     accum_out=totpsi[:, :])
                totpsiR = pool.tile([128, 2], BF16, tag="totpsiR")
                for c in range(2):
                    nc.vector.tensor_copy(totpsiR[:, c:c + 1], totpsi[:, :])
                cps = pp2.tile([128, 2], F32, tag="mm128", name="cps")
                nc.tensor.matmul(cps[:, :], CR[:, C_T128:C_T128 + 128],
                                 totpsiR[:, :], start=True, stop=True)
                aggraw = pool.tile([128, SEGW], F32, tag="aggraw")
                nc.vector.tensor_tensor_scan(
                    aggraw[:, :], encpsi[:, :], encpsi[:, :], 0.0,
                    op0=OP.add, op1=OP.bypass)
                agg = pool.tile([128, SEGW], BF16, tag="agg")
                nc.vector.scalar_tensor_tensor(
                    out=agg[:, :], in0=aggraw[:, :],
                    scalar=cps[:, 0:1], in1=CP[:, S_RC:S_RC + SEGW],
                    op0=OP.add, op1=OP.mult)
                if depth <= 3:
                    nc.sync.dma_start(out[:, 0:SEGW],
                                      agg[0:64, :].bitcast(F32))
                    return

                # ---------------- phase B: per-seg wide stages -------------
                enc4 = pool.tile([128, P], BF16, tag="enc4")
                w4 = pool.tile([128, P], BF16, tag="w4")
                X = pool.tile([128, P], BF16, tag="X")
                numl = pool.tile([128, P], BF16, tag="numl")
                denl = pool.tile([128, P], F32, tag="denl")
                rden = pool.tile([128, P], F32, tag="rden")
                out5 = pool.tile([128, P], BF16, tag="out5")
                hr1 = pool.tile([128, P // 2], BF16, tag="hr1")
                outT = pool.tile([128, P // 2], F32, tag="outT")

                pse4 = [pp2.tile([128, SEGW], F32, tag="pse4",
                                 name=f"pse4_{s}") for s in range(4)]
                s4 = [pp2.tile([128, SEGW], F32, tag="mm128",
                               name=f"s4_{s}") for s in range(4)]
                psr1 = [pp2.tile([64, SEGW], F32, tag="psr1",
                                 name=f"psr1_{s}") for s in range(4)]
                psr2 = [pp2.tile([128, SEGW], F32, tag="pse4",
                                 name=f"psr2_{p}") for p in range(2)]

                for s in range(4):
                    cs = segs[s]
                    g, pr = 32 * (s % 2), prs[s // 2]
                    j = s % 2
                    # logits+exp emitted BEFORE phi+enc4: exp gates the whole
                    # DVE den-scan train, enc4 only gates X/num
                    nc.tensor.matmul(s4[s][:, :],
                                     CR[0:64, C_WSA2 + 128 * j:
                                        C_WSA2 + 128 * (j + 1)],
                                     comb[0:64, pr],
                                     start=True, stop=False)
                    nc.tensor.matmul(s4[s][:, :],
                                     CR[:, C_WSF4 + 128 * s:
                                        C_WSF4 + 128 * (s + 1)],
                                     agg[:, :],
                                     start=False, stop=True)
                    # |logits| small for this model family: no max-shift
                    nc.scalar.activation(w4[:, cs], s4[s][:, :], AF.Exp)
                    nc.tensor.matmul(
                        pse4[s][:, :],
                        CR[:, C_PHI2 + 128 * j:C_PHI2 + 128 * (j + 1)],
                        h1[:, pr], start=True, stop=True)
                    nc.scalar.activation(enc4[:, cs], pse4[s][:, :], AF.Relu,
                                         bias=CP[:, S_BPHI4:S_BPHI4 + 1])
                    nc.gpsimd.tensor_tensor(X[:, cs], w4[:, cs],
                                             enc4[:, cs], OP.mult)
                    # scans chained across segs: numl/denl are globally
                    # cumulative, so no carry application is needed at all
                    initn = 0.0 if s == 0 else numl[:, s * SEGW - 1:s * SEGW]
                    nc.vector.tensor_tensor_scan(
                        numl[:, cs], X[:, cs], X[:, cs], initn,
                        op0=OP.add, op1=OP.bypass)
                    initd = 0.0 if s == 0 else denl[:, s * SEGW - 1:s * SEGW]
                    nc.vector.tensor_tensor_scan(
                        denl[:, cs], w4[:, cs], w4[:, cs], initd,
                        op0=OP.add, op1=OP.bypass)
                    nc.vector.reciprocal_approx_fast(rden[:, cs],
                                                     denl[:, cs])
                    o5eng = nc.gpsimd if s < 3 else nc.vector
                    o5eng.tensor_tensor(out5[:, cs], numl[:, cs],
                                        rden[:, cs], OP.mult)
                if depth <= 4:
                    nc.sync.dma_start(out[:, :], w4[0:64, :])
                    return

                # ---------------- rho MLP ---------------------------------
                for s in range(4):
                    nc.tensor.matmul(psr1[s][:, :], CR[:, C_R1:C_R1 + 64],
                                     out5[:, segs[s]], start=True, stop=True)
                    nc.scalar.activation(
                        hr1[64 * (s % 2):64 * (s % 2 + 1), prs[s // 2]],
                        psr1[s][:, :], AF.Relu,
                        bias=CP[0:64, S_RB1X2:S_RB1X2 + 1])
                    if s % 2 == 0:
                        continue
                    # emit each pair's tail right after its second hr1 so the
                    # scheduler keeps the chain-critical ops ahead
                    p = s // 2
                    nc.tensor.matmul(psr2[p][:, :], CR[:, C_R2B:C_R2B + 128],
                                     hr1[:, prs[p]], start=True, stop=True)
                    nc.scalar.activation(outT[:, prs[p]], psr2[p][:, :],
                                         AF.Relu,
                                         bias=CP[:, S_RB2X2:S_RB2X2 + 1])
                    for j in range(2):
                        eng = nc.sync if j == 0 else nc.scalar
                        eng.dma_start(
                            out[:, segs[2 * p + j]],
                            outT[64 * j:64 * (j + 1), prs[p]])

            if repeat == 1:
                body()
            else:
                with tc.For_i(0, repeat, 1):
                    body()

    nc.compile()
    return nc


def host_prep(inputs):
    """Fold parameters on the host; returns the packed const tensor."""
    f = lambda k: np.ascontiguousarray(np.asarray(inputs[k], np.float32))
    W_k, W_q = f("W_k"), f("W_q")
    Wq_exp = np.zeros((DOT * HEADS, HEADS), np.float32)
    for h in range(HEADS):
        for d in range(DOT):
            Wq_exp[d * HEADS + h, h] = W_q[h, d]
    Wpre = (W_k @ Wq_exp) / np.sqrt(np.float32(DOT))   # [63, 4]
    wpre_a = Wpre[:DIM_S]
    wfold = f("arho_w") @ Wpre[DIM_S:]                  # [32, 4]
    rep = np.repeat(np.arange(HEADS), PHI_W)            # [128]
    # comb row permutation: group order [pad, onehot22, values, tenc8]
    perm = np.concatenate([np.arange(9, 31), [8], np.arange(0, 8)])
    pad1 = lambda a: np.vstack([np.zeros((1, a.shape[1]), np.float32), a])
    wsa = pad1(np.ascontiguousarray(wpre_a[perm][:, rep]))   # [32, 128]
    wsf = np.ascontiguousarray(wfold[:, rep])           # [32, 128]
    w1p = pad1(np.ascontiguousarray(
        np.hstack([f("psi_w1"), f("phi_w1")])[perm]))   # [32, 64]

    psi2 = f("psi_w2")                                  # [32, 32]
    phi2rep = np.vstack([np.zeros((32, 128), np.float32),
                         np.tile(f("phi_w2"), (1, HEADS))])  # [64, 128]

    posvec = np.power(10000.0, 2.0 * (np.arange(D_TIME) // 2) / D_TIME)
    scale2pi = (1.0 / (posvec * 2 * np.pi)).astype(np.float32)
    shift2pi = np.where(np.arange(D_TIME) % 2 == 0, 0.0, 0.25).astype(
        np.float32)

    cp = np.zeros((128, NCW), np.float32)
    # W1X2: blockdiag over the comb pair groups -> h1 [even64 | odd64]
    cp[0:32, C_W1X2:C_W1X2 + 64] = w1p
    cp[32:64, C_W1X2 + 64:C_W1X2 + 128] = w1p
    # psi2 pieces: h1 rows 0..31 = psi-hidden even seg, 64..95 = odd seg;
    # pspsi rows 32s = seg s (s order 0,1 from pair0; 2,3 from pair1)
    cp[0:32, C_W2PL + 0:C_W2PL + 32] = psi2
    cp[64:96, C_W2PL + 32:C_W2PL + 64] = psi2
    cp[0:32, C_W2PH + 64:C_W2PH + 96] = psi2
    cp[64:96, C_W2PH + 96:C_W2PH + 128] = psi2
    # phi2 per parity: contract only the matching 64-row half of h1
    cp[0:64, C_PHI2:C_PHI2 + 128] = phi2rep
    cp[64:128, C_PHI2 + 128:C_PHI2 + 256] = phi2rep
    # wsa per parity: contract only the matching comb row group
    cp[0:32, C_WSA2:C_WSA2 + 128] = wsa
    cp[32:64, C_WSA2 + 128:C_WSA2 + 256] = wsa
    # wsf per seg: contract only agg partition group s
    for s in range(4):
        cp[32 * s:32 * (s + 1), C_WSF4 + 128 * s:C_WSF4 + 128 * (s + 1)] = wsf
    cp[:, C_R1:C_R1 + 64] = f("rho_w1")
    cp[0:64, C_R2B:C_R2B + 64] = f("rho_w2")
    cp[64:128, C_R2B + 64:C_R2B + 128] = f("rho_w2")
    t = np.zeros((128, 128), np.float32)
    for s in range(1, 4):
        for sp in range(s):
            t[np.arange(32) + 32 * sp, np.arange(32) + 32 * s] = 1.0
    cp[:, C_T128:C_T128 + 128] = t

    b1cat = np.concatenate([f("psi_b1"), f("phi_b1")])
    cp[:, CW_END + S_B1X2] = np.tile(b1cat, 2)
    cp[:, CW_END + S_BPSI4] = np.tile(f("psi_b2"), 4)
    cp[:, CW_END + S_BPHI4] = np.tile(f("phi_b2"), HEADS)
    cp[:, CW_END + S_RB1X2] = np.tile(f("rho_b1"), 2)
    cp[:, CW_END + S_RB2X2] = np.tile(f("rho_b2"), 2)
    cp[:, CW_END + S_SC] = np.repeat(scale2pi, NCH)
    cp[:, CW_END + S_SH] = np.repeat(shift2pi, NCH)
    # io: iota 1..22 in rows 1..22 / 33..54, -1 in the pad rows so the
    # one-hot compare zero-fills them
    cp[:, CW_END + S_IO] = -1.0
    io = np.arange(1, NUM_MODS + 1, dtype=np.float32)
    cp[1:1 + NUM_MODS, CW_END + S_IO] = io
    cp[33:33 + NUM_MODS, CW_END + S_IO] = io
    pos = np.arange(P, dtype=np.float32).reshape(4, SEGW)
    cp[:, CW_END + S_RC:CW_END + S_RC + SEGW] = np.repeat(
        1.0 / (pos + 1.0), 32, axis=0)
    import ml_dtypes
    cpw = np.ascontiguousarray(cp[:, 0:CW_END]).astype(ml_dtypes.bfloat16)
    cps = np.ascontiguousarray(cp[:, CW_END:NCW])
    return cpw, cps


def make_in_maps(inputs):
    cpw, cps = host_prep(inputs)
    times = np.asarray(inputs["times"], np.float32)
    values = np.asarray(inputs["values"], np.float32)
    meas = np.asarray(inputs["measurements"]).astype(np.float32)
    in_maps = []
    for b in range(B):
        in_maps.append({
            "cpw": cpw,
            "cps": cps,
            "times": np.ascontiguousarray(times[b][None, :]),
            "values": np.ascontiguousarray(values[b][None, :]),
            "meas": np.ascontiguousarray(meas[b][None, :]),
        })
    return in_maps


_NC_CACHE = {}


def _get_nc(repeat=1):
    if repeat not in _NC_CACHE:
        _NC_CACHE[repeat] = build(repeat)
    return _NC_CACHE[repeat]


def kernel(**inputs) -> np.ndarray:
    nc = _get_nc(1)
    in_maps = make_in_maps(inputs)
    res = bass_utils.run_bass_kernel_spmd(
        nc, in_maps, core_ids=list(range(N_CORES)))
    outs = [np.ascontiguousarray(res.results[b]["out"].T) for b in range(B)]
    return np.stack(outs, 0).astype(np.float32)


# revision 65
# speedup vs baseline: 1.0780x; 1.0780x over previous
"""Trainium2 Bass kernel for nn_DeepAttensionModule (cumulative set attention).

Self-contained: takes the FULL unsharded inputs of reference.setup_inputs(),
returns the FULL [4, 2048, 64] float32 output.

Strategy (v2)
-------------
Data-parallel over batch B=4: one NeuronCore per batch element (cores 0-3).
Per core, everything is channel-major [C, P=2048]; matmuls weight-stationary
fp32r.

Measured (wall-slope over a HW repeat loop, incl. per-iteration const
loads): ~42-45us/iter (min-min estimator over 10+ rounds; wall noise is
large), cost-model sim 33.8us, vs the 81.5us v1 harness baseline.

v2 changes vs v1 (81us baseline):
- ALL folded weights ship in ONE packed f32r const DMA + one f32 scalar DMA
  (v1 issued ~17 serialized 625ns DMAs on one queue).
- comb is built PAIR-PACKED [63, 1024]: partition group 0 holds even segs,
  group 1 odd segs; the one-hot is_equal writes straight into it.
- sin on DVE with a factored degree-7 polynomial after round-to-nearest
  range reduction, and 1/den via the single-instruction DVE
  reciprocal_approx_fast instead of ACT exp(-ln(den)) -> the whole kernel
  uses one activation table set (v1 thrashed ~10 x 1.3us table loads).
- everything in bf16 except the scans/reciprocal/carries (fp32) -- rel err
  6.9e-3 vs the 2e-2 gate.
- 32/64-row stages pack 2-4 segments across the 128 partitions via
  block-diagonal zero-padded lhsT (dst partition base always 0 -- PE column
  tiling is broken in this toolchain): psi MLP, the cumulative-psi scan, agg
  scale, and rho-2 each run as one or two [128,512] ops instead of 4 narrow
  ones. Cross-seg scan carries are applied for free in consumers (activation
  bias / scalar_tensor_tensor scalar); the agg carry uses a tiny 0/1-matrix
  matmul on PE.
- num/den scans chain across segs (init = previous last column) so they
  are globally cumulative: no carry application at all, and the normalize
  is a plain multiply. X/out5 multiplies run on Pool except the tail
  segment (DVE is faster; keeps the last-segment spine short).
- every PSUM tag is double-buffered in exactly 8 banks via tag sharing
  (carry-matmul rides the psi1 tag, rho-2 rides the phi2 tag), which
  un-serializes the rho-MLP tail.
"""
import numpy as np

import concourse.bacc as bacc
import concourse.mybir as mybir
import concourse.tile as tile
from concourse import bass_utils

B, P = 4, 2048
NUM_MODS, D_TIME = 22, 8
DIM_S = NUM_MODS + D_TIME + 1          # 31
PHI_W, PSI_W, PSI_LAT = 32, 32, 32
DOT, HEADS, RHO_W = 16, 4, 64
N_CORES = 4
SEGW = P // 4                           # 512
NCH = P // 128                          # 16 seq chunks in the [128,128] reshape

F32 = mybir.dt.float32
F32R = mybir.dt.float32r
BF16 = mybir.dt.bfloat16
I32 = mybir.dt.int32
AF = mybir.ActivationFunctionType
OP = mybir.AluOpType

# factored degree-7 odd minimax: sin(2*pi*d) ~= d*c7*(u-r1)*(u*u+pp*u+qq),
# u=d^2, |d|<=0.5, abs err 6.7e-4
SIN_C7 = -57.11540449516585
SIN_R1 = 0.24989525578673413
SIN_P = -1.1214834306739268
SIN_Q = 0.4399767340537331

# bf16 weights block columns (shipped as bf16 from the host -- no casting
# DMA, no f32r rounding constraints; PE speed is identical).
# ALL matmuls use lhsT/rhs/dst partition base 0: group selection is embedded
# as zero-padding in the weights (non-base-0 stationary loads hang the PE).
C_W1X2 = 0          # [64, 128] blockdiag(w1p, w1p); pad rows 31,63 zero
C_W2PL = 128        # [128, 128] psi2 blocks for h1 pair 0 -> pspsi rows 0..63
C_W2PH = 256        # [128, 128] psi2 blocks for h1 pair 1 -> rows 64..127
C_PHI2 = 384        # [128, 128] x2 parities: phi2 4x-rep, other half zero
C_WSA2 = 640        # [64, 128] x2 parities: folded logit weights (comb part)
C_WSF4 = 896        # [128, 128] x4 segs: folded logit weights (agg part)
C_R1 = 1408         # [128, 64]
C_R2B = 1472        # [128, 128] blockdiag(rho_w2, rho_w2)
C_T128 = 1600       # [128, 128] 0/1 exclusive-prefix-group matrix
CW_END = 1728
# f32 scalar block columns
S_B1X2 = 0          # [128, 1] psi_b1|phi_b1 twice
S_BPSI4 = 1         # [128, 1] psi_b2 4x
S_BPHI4 = 2         # [128, 1] phi_b2 4x
S_RB1X2 = 3         # [128, 1]
S_RB2X2 = 4         # [128, 1]
S_SC = 5            # [128, 1] tenc scale per reshape row
S_SH = 6            # [128, 1] tenc shift per reshape row
S_IO = 7            # iota 1..22 in partition groups 0 and 32
S_ZERO = 8          # all-zero column (pad-row fill source)
S_RC = 9            # [128, SEGW] 1/(512*s+c+1) in partition group s
NS = 9 + SEGW
NCW = CW_END + NS


def build(repeat: int = 1, depth: int = 99):
    nc = bacc.Bacc("TRN2", target_bir_lowering=False, debug=False,
                   num_devices=N_CORES)

    times = nc.dram_tensor("times", [1, P], F32, kind="ExternalInput").ap()
    values = nc.dram_tensor("values", [1, P], F32, kind="ExternalInput").ap()
    meas = nc.dram_tensor("meas", [1, P], F32, kind="ExternalInput").ap()
    cpw = nc.dram_tensor("cpw", [128, CW_END], BF16,
                         kind="ExternalInput").ap()
    cpsd = nc.dram_tensor("cps", [128, NS], F32,
                          kind="ExternalInput").ap()
    out = nc.dram_tensor("out", [RHO_W, P], F32, kind="ExternalOutput").ap()

    segs = [slice(s * SEGW, (s + 1) * SEGW) for s in range(4)]
    prs = [slice(p * SEGW, (p + 1) * SEGW) for p in range(2)]  # pair cols

    with tile.TileContext(nc) as tc:
        with tc.tile_pool(name="const", bufs=1) as cpool, \
             tc.tile_pool(name="work", bufs=1) as pool, \
             tc.tile_pool(name="psum", bufs=1, space="PSUM") as pp, \
             tc.tile_pool(name="psum2", bufs=2, space="PSUM") as pp2, \
             tc.tile_pool(name="dram", bufs=1, space="DRAM") as dp:

            CR = cpool.tile([128, CW_END], BF16, tag="cpackw")
            CP = cpool.tile([128, NS], F32, tag="cpacks")
            state = {"first": True}

            def body():
                # ---------------- comb assembly (pair-packed) --------------
                # row group layout (x2 at base 0/32 for even/odd seg of the
                # pair): [pad, onehot*22, values, tenc*8] = 32 rows.
                # columns: pair 0 = segs 0,1 | pair 1 = segs 2,3
                comb = pool.tile([64, P // 2], BF16, tag="comb")

                # tenc source load first (gates the longest front-end chain)
                t128 = pool.tile([128, 128], F32, tag="t128")
                nc.sync.dma_start(
                    t128[:, :],
                    times.rearrange("o (k i) -> o k i", i=128).broadcast_to(
                        [8, NCH, 128]))
                # values rows 23 / 55 via casting SWDGE DMAs straight from
                # DRAM -- idle Pool queue, ready long before comb is needed
                vview = values.rearrange("o (p j c) -> o j p c", p=2, j=2)
                for jj in range(2):
                    nc.gpsimd.dma_start(comb[32 * jj + 23:32 * jj + 24, :],
                                        vview[:, jj])
                if state["first"]:
                    # hot scalar columns first (they gate the tenc chain);
                    # the RC block and the weight pack can land later
                    nc.scalar.dma_start(CP[:, 0:8], cpsd[:, 0:8])
                    state["first"] = False

                mb = pool.tile([32, P], F32, tag="mb")
                nc.scalar.dma_start(mb[:, :], meas.broadcast_to([32, P]))


                # tenc rows 23..30 / 55..62
                q = pool.tile([128, 128], F32, tag="q")
                nc.vector.tensor_scalar(q[:, :], t128[:, :],
                                        CP[:, S_SC:S_SC + 1],
                                        CP[:, S_SH:S_SH + 1],
                                        OP.mult, OP.add)
                ni = pool.tile([128, 128], I32, tag="ni")
                nc.vector.tensor_copy(ni[:, :], q[:, :])
                nf = pool.tile([128, 128], F32, tag="nf")
                nc.vector.tensor_copy(nf[:, :], ni[:, :])
                d8 = pool.tile([128, 128], F32, tag="d8")
                nc.vector.scalar_tensor_tensor(
                    out=d8[:, :], in0=nf[:, :], scalar=-1.0, in1=q[:, :],
                    op0=OP.mult, op1=OP.add)
                # sin(2*pi*d) = (((u-r1)*d) * ((u+pp)*u+qq)) * c7
                uu = pool.tile([128, 128], F32, tag="uu")
                nc.vector.tensor_tensor(uu[:, :], d8[:, :], d8[:, :], OP.mult)
                t1 = pool.tile([128, 128], F32, tag="t1")
                nc.vector.scalar_tensor_tensor(
                    out=t1[:, :], in0=uu[:, :], scalar=-SIN_R1, in1=d8[:, :],
                    op0=OP.add, op1=OP.mult)
                vv = pool.tile([128, 128], F32, tag="vv")
                nc.vector.scalar_tensor_tensor(
                    out=vv[:, :], in0=uu[:, :], scalar=SIN_P, in1=uu[:, :],
                    op0=OP.add, op1=OP.mult)
                ww = pool.tile([128, 128], F32, tag="ww")
                nc.vector.scalar_tensor_tensor(
                    out=ww[:, :], in0=vv[:, :], scalar=SIN_Q, in1=t1[:, :],
                    op0=OP.add, op1=OP.mult)
                tsin = pool.tile([128, 128], BF16, tag="tsin")
                nc.vector.tensor_scalar(tsin[:, :], ww[:, :], SIN_C7, None,
                                        OP.mult)
                # relayout via f32r DRAM bounce (SBUF-src DMAs cannot walk
                # multi-level partition patterns; DRAM-src can). tdram
                # partition index is j*16+p*8+jj*4+k (j=tenc row, p=pair,
                # jj=parity, k=chunk), payload col i -> comb col 512p+128k+i
                tdram = dp.tile([128, 128], BF16, tag="tdram")
                nc.sync.dma_start(tdram[:, :], tsin[:, :])
                tgat = tdram[:, :].rearrange("(j p jj k) i -> jj j p k i",
                                             j=8, p=2, jj=2, k=4)
                nc.sync.dma_start(comb[24:32, :], tgat[0])
                nc.scalar.dma_start(comb[56:64, :], tgat[1])

                # pad + one-hot rows g+0..g+22 (emitted after the tenc chain
                # so the DVE runs the latency-critical sin path first); io row
                # g+0 is -1 so the compare zero-fills the pad row. No write
                # overlaps values/tenc rows -> order-free.
                for s in range(4):
                    g, pr = 32 * (s % 2), prs[s // 2]
                    nc.vector.tensor_scalar(
                        comb[g:g + 23, pr], mb[0:23, segs[s]],
                        CP[g:g + 23, S_IO:S_IO + 1], None, OP.is_equal)

                if state.get("cr", True):
                    nc.sync.dma_start(CR[:, :], cpw)
                    nc.scalar.dma_start(CP[:, 8:NS], cpsd[:, 8:NS])
                    state["cr"] = False

                if depth <= 1:
                    nc.sync.dma_start(out[:, 0:P // 2],
                                      comb[:, :].bitcast(F32))
                    return

                # ---------------- phase A: psi branch (packed) -------------
                h1 = pool.tile([128, P // 2], BF16, tag="h1")  # pair-packed

                ps1 = [pp2.tile([128, SEGW], F32, tag="ps1",
                                name=f"ps1_{p}") for p in range(2)]
                for p in range(2):
                    nc.tensor.matmul(ps1[p][:, :],
                                     CR[0:64, C_W1X2:C_W1X2 + 128],
                                     comb[0:64, prs[p]],
                                     start=True, stop=True)
                    nc.scalar.activation(h1[:, prs[p]], ps1[p][:, :],
                                         AF.Relu,
                                         bias=CP[:, S_B1X2:S_B1X2 + 1])
                if depth <= 2:
                    nc.sync.dma_start(out[:, 0:P // 2],
                                      h1[0:64, :].bitcast(F32))
                    return

                # psi2: 4x32 partition-packed via two accumulating matmuls
                pspsi = pp2.tile([128, SEGW], F32, tag="mm128",
                                 name="pspsi")
                nc.tensor.matmul(pspsi[:, :], CR[:, C_W2PL:C_W2PL + 128],
                                 h1[:, prs[0]], start=True, stop=False)
                nc.tensor.matmul(pspsi[:, :], CR[:, C_W2PH:C_W2PH + 128],
                                 h1[:, prs[1]], start=False, stop=True)
                encpsi = pool.tile([128, SEGW], F32, tag="encpsi")
                totpsi = pool.tile([128, 1], F32, tag="totpsi")
                nc.scalar.activation(encpsi[:, :], pspsi[:, :], AF.Relu,
                                     bias=CP[:, S_BPSI4:S_BPSI4 + 1],
                                     accum_out=totpsi[:, :])
                totpsiR = pool.tile([128, 2], BF16, tag="totpsiR")
                for c in range(2):
                    nc.vector.tensor_copy(totpsiR[:, c:c + 1], totpsi[:, :])
                cps = pp2.tile([128, 2], F32, tag="mm128", name="cps")
                nc.tensor.matmul(cps[:, :], CR[:, C_T128:C_T128 + 128],
                                 totpsiR[:, :], start=True, stop=True)
                aggraw = pool.tile([128, SEGW], F32, tag="aggraw")
                nc.vector.tensor_tensor_scan(
                    aggraw[:, :], encpsi[:, :], encpsi[:, :], 0.0,
                    op0=OP.add, op1=OP.bypass)
                agg = pool.tile([128, SEGW], BF16, tag="agg")
                nc.vector.scalar_tensor_tensor(
                    out=agg[:, :], in0=aggraw[:, :],
                    scalar=cps[:, 0:1], in1=CP[:, S_RC:S_RC + SEGW],
                    op0=OP.add, op1=OP.mult)
                if depth <= 3:
                    nc.sync.dma_start(out[:, 0:SEGW],
                                      agg[0:64, :].bitcast(F32))
                    return

                # ---------------- phase B: per-seg wide stages -------------
                enc4 = pool.tile([128, P], BF16, tag="enc4")
                w4 = pool.tile([128, P], BF16, tag="w4")
                X = pool.tile([128, P], BF16, tag="X")
                numl = pool.tile([128, P], BF16, tag="numl")
                denl = pool.tile([128, P], F32, tag="denl")
                rden = pool.tile([128, P], F32, tag="rden")
                out5 = pool.tile([128, P], BF16, tag="out5")
                hr1 = pool.tile([128, P // 2], BF16, tag="hr1")
                outT = pool.tile([128, P // 2], F32, tag="outT")

                pse4 = [pp2.tile([128, SEGW], F32, tag="pse4",
                                 name=f"pse4_{s}") for s in range(4)]
                s4 = [pp2.tile([128, SEGW], F32, tag="mm128",
                               name=f"s4_{s}") for s in range(4)]
                psr1 = [pp2.tile([64, SEGW], F32, tag="psr1",
                                 name=f"psr1_{s}") for s in range(4)]
                psr2 = [pp2.tile([128, SEGW], F32, tag="pse4",
                                 name=f"psr2_{p}") for p in range(2)]

                for s in range(4):
                    cs = segs[s]
                    g, pr = 32 * (s % 2), prs[s // 2]
                    j = s % 2
                    nc.tensor.matmul(
                        pse4[s][:, :],
                        CR[:, C_PHI2 + 128 * j:C_PHI2 + 128 * (j + 1)],
                        h1[:, pr], start=True, stop=True)
                    nc.scalar.activation(enc4[:, cs], pse4[s][:, :], AF.Relu,
                                         bias=CP[:, S_BPHI4:S_BPHI4 + 1])
                    nc.tensor.matmul(s4[s][:, :],
                                     CR[0:64, C_WSA2 + 128 * j:
                                        C_WSA2 + 128 * (j + 1)],
                                     comb[0:64, pr],
                                     start=True, stop=False)
                    nc.tensor.matmul(s4[s][:, :],
                                     CR[:, C_WSF4 + 128 * s:
                                        C_WSF4 + 128 * (s + 1)],
                                     agg[:, :],
                                     start=False, stop=True)
                    # |logits| small for this model family: no max-shift
                    nc.scalar.activation(w4[:, cs], s4[s][:, :], AF.Exp)
                    xeng = nc.gpsimd if s < 3 else nc.vector
                    xeng.tensor_tensor(X[:, cs], w4[:, cs],
                                       enc4[:, cs], OP.mult)
                    # scans chained across segs: numl/denl are globally
                    # cumulative, so no carry application is needed at all
                    initn = 0.0 if s == 0 else numl[:, s * SEGW - 1:s * SEGW]
                    nc.vector.tensor_tensor_scan(
                        numl[:, cs], X[:, cs], X[:, cs], initn,
                        op0=OP.add, op1=OP.bypass)
                    initd = 0.0 if s == 0 else denl[:, s * SEGW - 1:s * SEGW]
                    nc.vector.tensor_tensor_scan(
                        denl[:, cs], w4[:, cs], w4[:, cs], initd,
                        op0=OP.add, op1=OP.bypass)
                    nc.vector.reciprocal_approx_fast(rden[:, cs],
                                                     denl[:, cs])
                    o5eng = nc.gpsimd if s < 3 else nc.vector
                    o5eng.tensor_tensor(out5[:, cs], numl[:, cs],
                                        rden[:, cs], OP.mult)
                if depth <= 4:
                    nc.sync.dma_start(out[:, :], w4[0:64, :])
                    return

                # ---------------- rho MLP ---------------------------------
                for s in range(4):
                    nc.tensor.matmul(psr1[s][:, :], CR[:, C_R1:C_R1 + 64],
                                     out5[:, segs[s]], start=True, stop=True)
                    nc.scalar.activation(
                        hr1[64 * (s % 2):64 * (s % 2 + 1), prs[s // 2]],
                        psr1[s][:, :], AF.Relu,
                        bias=CP[0:64, S_RB1X2:S_RB1X2 + 1])
                    if s % 2 == 0:
                        continue
                    # emit each pair's tail right after its second hr1 so the
                    # scheduler keeps the chain-critical ops ahead
                    p = s // 2
                    nc.tensor.matmul(psr2[p][:, :], CR[:, C_R2B:C_R2B + 128],
                                     hr1[:, prs[p]], start=True, stop=True)
                    nc.scalar.activation(outT[:, prs[p]], psr2[p][:, :],
                                         AF.Relu,
                                         bias=CP[:, S_RB2X2:S_RB2X2 + 1])
                    for j in range(2):
                        eng = nc.sync if j == 0 else nc.scalar
                        eng.dma_start(
                            out[:, segs[2 * p + j]],
                            outT[64 * j:64 * (j + 1), prs[p]])

            if repeat == 1:
                body()
            else:
                with tc.For_i(0, repeat, 1):
                    body()

    nc.compile()
    return nc


def host_prep(inputs):
    """Fold parameters on the host; returns the packed const tensor."""
    f = lambda k: np.ascontiguousarray(np.asarray(inputs[k], np.float32))
    W_k, W_q = f("W_k"), f("W_q")
    Wq_exp = np.zeros((DOT * HEADS, HEADS), np.float32)
    for h in range(HEADS):
        for d in range(DOT):
            Wq_exp[d * HEADS + h, h] = W_q[h, d]
    Wpre = (W_k @ Wq_exp) / np.sqrt(np.float32(DOT))   # [63, 4]
    wpre_a = Wpre[:DIM_S]
    wfold = f("arho_w") @ Wpre[DIM_S:]                  # [32, 4]
    rep = np.repeat(np.arange(HEADS), PHI_W)            # [128]
    # comb row permutation: group order [pad, onehot22, values, tenc8]
    perm = np.concatenate([np.arange(9, 31), [8], np.arange(0, 8)])
    pad1 = lambda a: np.vstack([np.zeros((1, a.shape[1]), np.float32), a])
    wsa = pad1(np.ascontiguousarray(wpre_a[perm][:, rep]))   # [32, 128]
    wsf = np.ascontiguousarray(wfold[:, rep])           # [32, 128]
    w1p = pad1(np.ascontiguousarray(
        np.hstack([f("psi_w1"), f("phi_w1")])[perm]))   # [32, 64]

    psi2 = f("psi_w2")                                  # [32, 32]
    phi2rep = np.vstack([np.zeros((32, 128), np.float32),
                         np.tile(f("phi_w2"), (1, HEADS))])  # [64, 128]

    posvec = np.power(10000.0, 2.0 * (np.arange(D_TIME) // 2) / D_TIME)
    scale2pi = (1.0 / (posvec * 2 * np.pi)).astype(np.float32)
    shift2pi = np.where(np.arange(D_TIME) % 2 == 0, 0.0, 0.25).astype(
        np.float32)

    cp = np.zeros((128, NCW), np.float32)
    # W1X2: blockdiag over the comb pair groups -> h1 [even64 | odd64]
    cp[0:32, C_W1X2:C_W1X2 + 64] = w1p
    cp[32:64, C_W1X2 + 64:C_W1X2 + 128] = w1p
    # psi2 pieces: h1 rows 0..31 = psi-hidden even seg, 64..95 = odd seg;
    # pspsi rows 32s = seg s (s order 0,1 from pair0; 2,3 from pair1)
    cp[0:32, C_W2PL + 0:C_W2PL + 32] = psi2
    cp[64:96, C_W2PL + 32:C_W2PL + 64] = psi2
    cp[0:32, C_W2PH + 64:C_W2PH + 96] = psi2
    cp[64:96, C_W2PH + 96:C_W2PH + 128] = psi2
    # phi2 per parity: contract only the matching 64-row half of h1
    cp[0:64, C_PHI2:C_PHI2 + 128] = phi2rep
    cp[64:128, C_PHI2 + 128:C_PHI2 + 256] = phi2rep
    # wsa per parity: contract only the matching comb row group
    cp[0:32, C_WSA2:C_WSA2 + 128] = wsa
    cp[32:64, C_WSA2 + 128:C_WSA2 + 256] = wsa
    # wsf per seg: contract only agg partition group s
    for s in range(4):
        cp[32 * s:32 * (s + 1), C_WSF4 + 128 * s:C_WSF4 + 128 * (s + 1)] = wsf
    cp[:, C_R1:C_R1 + 64] = f("rho_w1")
    cp[0:64, C_R2B:C_R2B + 64] = f("rho_w2")
    cp[64:128, C_R2B + 64:C_R2B + 128] = f("rho_w2")
    t = np.zeros((128, 128), np.float32)
    for s in range(1, 4):
        for sp in range(s):
            t[np.arange(32) + 32 * sp, np.arange(32) + 32 * s] = 1.0
    cp[:, C_T128:C_T128 + 128] = t

    b1cat = np.concatenate([f("psi_b1"), f("phi_b1")])
    cp[:, CW_END + S_B1X2] = np.tile(b1cat, 2)
    cp[:, CW_END + S_BPSI4] = np.tile(f("psi_b2"), 4)
    cp[:, CW_END + S_BPHI4] = np.tile(f("phi_b2"), HEADS)
    cp[:, CW_END + S_RB1X2] = np.tile(f("rho_b1"), 2)
    cp[:, CW_END + S_RB2X2] = np.tile(f("rho_b2"), 2)
    cp[:, CW_END + S_SC] = np.repeat(scale2pi, NCH)
    cp[:, CW_END + S_SH] = np.repeat(shift2pi, NCH)
    # io: iota 1..22 in rows 1..22 / 33..54, -1 in the pad rows so the
    # one-hot compare zero-fills them
    cp[:, CW_END + S_IO] = -1.0
    io = np.arange(1, NUM_MODS + 1, dtype=np.float32)
    cp[1:1 + NUM_MODS, CW_END + S_IO] = io
    cp[33:33 + NUM_MODS, CW_END + S_IO] = io
    pos = np.arange(P, dtype=np.float32).reshape(4, SEGW)
    cp[:, CW_END + S_RC:CW_END + S_RC + SEGW] = np.repeat(
        1.0 / (pos + 1.0), 32, axis=0)
    import ml_dtypes
    cpw = np.ascontiguousarray(cp[:, 0:CW_END]).astype(ml_dtypes.bfloat16)
    cps = np.ascontiguousarray(cp[:, CW_END:NCW])
    return cpw, cps


def make_in_maps(inputs):
    cpw, cps = host_prep(inputs)
    times = np.asarray(inputs["times"], np.float32)
    values = np.asarray(inputs["values"], np.float32)
    meas = np.asarray(inputs["measurements"]).astype(np.float32)
    in_maps = []
    for b in range(B):
        in_maps.append({
            "cpw": cpw,
            "cps": cps,
            "times": np.ascontiguousarray(times[b][None, :]),
            "values": np.ascontiguousarray(values[b][None, :]),
            "meas": np.ascontiguousarray(meas[b][None, :]),
        })
    return in_maps


_NC_CACHE = {}


def _get_nc(repeat=1):
    if repeat not in _NC_CACHE:
        _NC_CACHE[repeat] = build(repeat)
    return _NC_CACHE[repeat]


def kernel(**inputs) -> np.ndarray:
    nc = _get_nc(1)
    in_maps = make_in_maps(inputs)
    res = bass_utils.run_bass_kernel_spmd(
        nc, in_maps, core_ids=list(range(N_CORES)))
    outs = [np.ascontiguousarray(res.results[b]["out"].T) for b in range(B)]
    return np.stack(outs, 0).astype(np.float32)


# revision 72
# speedup vs baseline: 1.1165x; 1.0357x over previous
"""Trainium2 Bass kernel for nn_DeepAttensionModule (cumulative set attention).

Self-contained: takes the FULL unsharded inputs of reference.setup_inputs(),
returns the FULL [4, 2048, 64] float32 output.

Strategy (v2)
-------------
Data-parallel over batch B=4: one NeuronCore per batch element (cores 0-3).
Per core, everything is channel-major [C, P=2048]; matmuls weight-stationary
fp32r.

Measured (wall-slope over a HW repeat loop, incl. per-iteration const
loads): ~44us/iter (min-min estimator over 12 rounds; wall noise is
large), cost-model sim 33.6us, vs the 81.5us v1 harness baseline.

v2 changes vs v1 (81us baseline):
- ALL folded weights ship in ONE packed f32r const DMA + one f32 scalar DMA
  (v1 issued ~17 serialized 625ns DMAs on one queue).
- comb is built PAIR-PACKED [63, 1024]: partition group 0 holds even segs,
  group 1 odd segs; the one-hot is_equal writes straight into it.
- sin on DVE with a factored degree-7 polynomial after round-to-nearest
  range reduction, and 1/den via the single-instruction DVE
  reciprocal_approx_fast instead of ACT exp(-ln(den)) -> the whole kernel
  uses one activation table set (v1 thrashed ~10 x 1.3us table loads).
- everything in bf16 except the scans/reciprocal/carries (fp32) -- rel err
  6.9e-3 vs the 2e-2 gate.
- 32/64-row stages pack 2-4 segments across the 128 partitions via
  block-diagonal zero-padded lhsT (dst partition base always 0 -- PE column
  tiling is broken in this toolchain): psi MLP, the cumulative-psi scan, agg
  scale, and rho-2 each run as one or two [128,512] ops instead of 4 narrow
  ones. Cross-seg scan carries are applied for free in consumers (activation
  bias / scalar_tensor_tensor scalar); the agg carry uses a tiny 0/1-matrix
  matmul on PE.
- num/den scans chain across segs (init = previous last column) so they
  are globally cumulative: no carry application at all, and the normalize
  is a plain multiply. X/out5 multiplies run on Pool except the tail
  segment (DVE is faster; keeps the last-segment spine short).
- every PSUM tag is double-buffered in exactly 8 banks via tag sharing
  (carry-matmul rides the psi1 tag, rho-2 rides the phi2 tag), which
  un-serializes the rho-MLP tail.
"""
import numpy as np

import concourse.bacc as bacc
import concourse.mybir as mybir
import concourse.tile as tile
from concourse import bass_utils

B, P = 4, 2048
NUM_MODS, D_TIME = 22, 8
DIM_S = NUM_MODS + D_TIME + 1          # 31
PHI_W, PSI_W, PSI_LAT = 32, 32, 32
DOT, HEADS, RHO_W = 16, 4, 64
N_CORES = 4
SEGW = P // 4                           # 512
NCH = P // 128                          # 16 seq chunks in the [128,128] reshape

F32 = mybir.dt.float32
F32R = mybir.dt.float32r
BF16 = mybir.dt.bfloat16
I32 = mybir.dt.int32
AF = mybir.ActivationFunctionType
OP = mybir.AluOpType

# factored degree-7 odd minimax: sin(2*pi*d) ~= d*c7*(u-r1)*(u*u+pp*u+qq),
# u=d^2, |d|<=0.5, abs err 6.7e-4
SIN_C7 = -57.11540449516585
SIN_R1 = 0.24989525578673413
SIN_P = -1.1214834306739268
SIN_Q = 0.4399767340537331

# bf16 weights block columns (shipped as bf16 from the host -- no casting
# DMA, no f32r rounding constraints; PE speed is identical).
# ALL matmuls use lhsT/rhs/dst partition base 0: group selection is embedded
# as zero-padding in the weights (non-base-0 stationary loads hang the PE).
C_W1X2 = 0          # [64, 128] blockdiag(w1p, w1p); pad rows 31,63 zero
C_W2PL = 128        # [128, 128] psi2 blocks for h1 pair 0 -> pspsi rows 0..63
C_W2PH = 256        # [128, 128] psi2 blocks for h1 pair 1 -> rows 64..127
C_PHI2 = 384        # [128, 128] x2 parities: phi2 4x-rep, other half zero
C_WSA2 = 640        # [64, 128] x2 parities: folded logit weights (comb part)
C_WSF4 = 896        # [128, 128] x4 segs: folded logit weights (agg part)
C_R1 = 1408         # [128, 64]
C_R2B = 1472        # [128, 128] blockdiag(rho_w2, rho_w2)
C_T128 = 1600       # [128, 128] 0/1 exclusive-prefix-group matrix
CW_END = 1728
# f32 scalar block columns
S_B1X2 = 0          # [128, 1] psi_b1|phi_b1 twice
S_BPSI4 = 1         # [128, 1] psi_b2 4x
S_BPHI4 = 2         # [128, 1] phi_b2 4x
S_RB1X2 = 3         # [128, 1]
S_RB2X2 = 4         # [128, 1]
S_SC = 5            # [128, 1] tenc scale per reshape row
S_SH = 6            # [128, 1] tenc shift per reshape row
S_IO = 7            # iota 1..22 in partition groups 0 and 32
S_ZERO = 8          # all-zero column (pad-row fill source)
S_RC = 9            # [128, SEGW] 1/(512*s+c+1) in partition group s
NS = 9 + SEGW
NCW = CW_END + NS


def build(repeat: int = 1, depth: int = 99):
    nc = bacc.Bacc("TRN2", target_bir_lowering=False, debug=False,
                   num_devices=N_CORES)

    times = nc.dram_tensor("times", [1, P], F32, kind="ExternalInput").ap()
    values = nc.dram_tensor("values", [1, P], F32, kind="ExternalInput").ap()
    meas = nc.dram_tensor("meas", [1, P], F32, kind="ExternalInput").ap()
    cpw = nc.dram_tensor("cpw", [128, CW_END], BF16,
                         kind="ExternalInput").ap()
    cpsd = nc.dram_tensor("cps", [128, NS], F32,
                          kind="ExternalInput").ap()
    out = nc.dram_tensor("out", [RHO_W, P], F32, kind="ExternalOutput").ap()

    segs = [slice(s * SEGW, (s + 1) * SEGW) for s in range(4)]
    prs = [slice(p * SEGW, (p + 1) * SEGW) for p in range(2)]  # pair cols

    with tile.TileContext(nc) as tc:
        with tc.tile_pool(name="const", bufs=1) as cpool, \
             tc.tile_pool(name="work", bufs=1) as pool, \
             tc.tile_pool(name="psum", bufs=1, space="PSUM") as pp, \
             tc.tile_pool(name="psum2", bufs=2, space="PSUM") as pp2, \
             tc.tile_pool(name="dram", bufs=1, space="DRAM") as dp:

            CR = cpool.tile([128, CW_END], BF16, tag="cpackw")
            CP = cpool.tile([128, NS], F32, tag="cpacks")
            state = {"first": True}

            def body():
                # ---------------- comb assembly (pair-packed) --------------
                # row group layout (x2 at base 0/32 for even/odd seg of the
                # pair): [pad, onehot*22, values, tenc*8] = 32 rows.
                # columns: pair 0 = segs 0,1 | pair 1 = segs 2,3
                comb = pool.tile([64, P // 2], BF16, tag="comb")

                # tenc source load first (gates the longest front-end chain)
                t128 = pool.tile([128, 128], F32, tag="t128")
                nc.sync.dma_start(
                    t128[:, :],
                    times.rearrange("o (k i) -> o k i", i=128).broadcast_to(
                        [8, NCH, 128]))
                # values rows 23 / 55 via casting SWDGE DMAs straight from
                # DRAM -- idle Pool queue, ready long before comb is needed
                vview = values.rearrange("o (p j c) -> o j p c", p=2, j=2)
                for jj in range(2):
                    nc.gpsimd.dma_start(comb[32 * jj + 23:32 * jj + 24, :],
                                        vview[:, jj])
                if state["first"]:
                    # hot scalar columns first (they gate the tenc chain);
                    # the RC block and the weight pack can land later
                    nc.scalar.dma_start(CP[:, 0:8], cpsd[:, 0:8])
                    state["first"] = False

                mb = pool.tile([32, P], F32, tag="mb")
                nc.scalar.dma_start(mb[:, :], meas.broadcast_to([32, P]))


                # tenc rows 23..30 / 55..62
                q = pool.tile([128, 128], F32, tag="q")
                nc.vector.tensor_scalar(q[:, :], t128[:, :],
                                        CP[:, S_SC:S_SC + 1],
                                        CP[:, S_SH:S_SH + 1],
                                        OP.mult, OP.add)
                ni = pool.tile([128, 128], I32, tag="ni")
                nc.vector.tensor_copy(ni[:, :], q[:, :])
                nf = pool.tile([128, 128], F32, tag="nf")
                nc.vector.tensor_copy(nf[:, :], ni[:, :])
                d8 = pool.tile([128, 128], F32, tag="d8")
                nc.vector.scalar_tensor_tensor(
                    out=d8[:, :], in0=nf[:, :], scalar=-1.0, in1=q[:, :],
                    op0=OP.mult, op1=OP.add)
                # sin(2*pi*d) = (((u-r1)*d) * ((u+pp)*u+qq)) * c7
                uu = pool.tile([128, 128], F32, tag="uu")
                nc.vector.tensor_tensor(uu[:, :], d8[:, :], d8[:, :], OP.mult)
                # t1 = c7*(u - r1)*d via the fused affine-mul custom op,
                # folding the final *c7 scale away (accum is a mandatory
                # scratch output, unused)
                t1 = pool.tile([128, 128], F32, tag="t1")
                tacc = pool.tile([128, 1], F32, tag="tacc")
                nc.vector.affine_mul_reduce(t1[:, :], tacc[:, :], uu[:, :],
                                            d8[:, :], SIN_C7,
                                            -SIN_R1 * SIN_C7)
                vv = pool.tile([128, 128], F32, tag="vv")
                nc.vector.scalar_tensor_tensor(
                    out=vv[:, :], in0=uu[:, :], scalar=SIN_P, in1=uu[:, :],
                    op0=OP.add, op1=OP.mult)

                tsin = pool.tile([128, 128], BF16, tag="tsin")
                nc.vector.scalar_tensor_tensor(
                    out=tsin[:, :], in0=vv[:, :], scalar=SIN_Q, in1=t1[:, :],
                    op0=OP.add, op1=OP.mult)
                # relayout via f32r DRAM bounce (SBUF-src DMAs cannot walk
                # multi-level partition patterns; DRAM-src can). tdram
                # partition index is j*16+p*8+jj*4+k (j=tenc row, p=pair,
                # jj=parity, k=chunk), payload col i -> comb col 512p+128k+i
                tdram = dp.tile([128, 128], BF16, tag="tdram")
                nc.sync.dma_start(tdram[:, :], tsin[:, :])
                tgat = tdram[:, :].rearrange("(j p jj k) i -> jj j p k i",
                                             j=8, p=2, jj=2, k=4)
                nc.sync.dma_start(comb[24:32, :], tgat[0])
                nc.scalar.dma_start(comb[56:64, :], tgat[1])

                # pad + one-hot rows g+0..g+22 (emitted after the tenc chain
                # so the DVE runs the latency-critical sin path first); io row
                # g+0 is -1 so the compare zero-fills the pad row. No write
                # overlaps values/tenc rows -> order-free.
                for s in range(4):
                    g, pr = 32 * (s % 2), prs[s // 2]
                    nc.vector.tensor_scalar(
                        comb[g:g + 23, pr], mb[0:23, segs[s]],
                        CP[g:g + 23, S_IO:S_IO + 1], None, OP.is_equal)

                if state.get("cr", True):
                    nc.sync.dma_start(CR[:, :], cpw)
                    nc.scalar.dma_start(CP[:, 8:NS], cpsd[:, 8:NS])
                    state["cr"] = False

                if depth <= 1:
                    nc.sync.dma_start(out[:, 0:P // 2],
                                      comb[:, :].bitcast(F32))
                    return

                # ---------------- phase A: psi branch (packed) -------------
                h1 = pool.tile([128, P // 2], BF16, tag="h1")  # pair-packed

                ps1 = [pp2.tile([128, SEGW], F32, tag="ps1",
                                name=f"ps1_{p}") for p in range(2)]
                for p in range(2):
                    nc.tensor.matmul(ps1[p][:, :],
                                     CR[0:64, C_W1X2:C_W1X2 + 128],
                                     comb[0:64, prs[p]],
                                     start=True, stop=True)
                    nc.scalar.activation(h1[:, prs[p]], ps1[p][:, :],
                                         AF.Relu,
                                         bias=CP[:, S_B1X2:S_B1X2 + 1])
                if depth <= 2:
                    nc.sync.dma_start(out[:, 0:P // 2],
                                      h1[0:64, :].bitcast(F32))
                    return

                # psi2: 4x32 partition-packed via two accumulating matmuls
                pspsi = pp2.tile([128, SEGW], F32, tag="mm128",
                                 name="pspsi")
                nc.tensor.matmul(pspsi[:, :], CR[:, C_W2PL:C_W2PL + 128],
                                 h1[:, prs[0]], start=True, stop=False)
                nc.tensor.matmul(pspsi[:, :], CR[:, C_W2PH:C_W2PH + 128],
                                 h1[:, prs[1]], start=False, stop=True)
                encpsi = pool.tile([128, SEGW], F32, tag="encpsi")
                totpsi = pool.tile([128, 1], F32, tag="totpsi")
                nc.scalar.activation(encpsi[:, :], pspsi[:, :], AF.Relu,
                                     bias=CP[:, S_BPSI4:S_BPSI4 + 1],
                                     accum_out=totpsi[:, :])
                totpsiR = pool.tile([128, 2], BF16, tag="totpsiR")
                for c in range(2):
                    nc.vector.tensor_copy(totpsiR[:, c:c + 1], totpsi[:, :])
                cps = pp2.tile([128, 2], F32, tag="mm128", name="cps")
                nc.tensor.matmul(cps[:, :], CR[:, C_T128:C_T128 + 128],
                                 totpsiR[:, :], start=True, stop=True)
                aggraw = pool.tile([128, SEGW], F32, tag="aggraw")
                nc.vector.tensor_tensor_scan(
                    aggraw[:, :], encpsi[:, :], encpsi[:, :], 0.0,
                    op0=OP.add, op1=OP.bypass)
                agg = pool.tile([128, SEGW], BF16, tag="agg")
                nc.vector.scalar_tensor_tensor(
                    out=agg[:, :], in0=aggraw[:, :],
                    scalar=cps[:, 0:1], in1=CP[:, S_RC:S_RC + SEGW],
                    op0=OP.add, op1=OP.mult)
                if depth <= 3:
                    nc.sync.dma_start(out[:, 0:SEGW],
                                      agg[0:64, :].bitcast(F32))
                    return

                # ---------------- phase B: per-seg wide stages -------------
                enc4 = pool.tile([128, P], BF16, tag="enc4")
                w4 = pool.tile([128, P], BF16, tag="w4")
                X = pool.tile([128, P], BF16, tag="X")
                numl = pool.tile([128, P], BF16, tag="numl")
                denl = pool.tile([128, P], F32, tag="denl")
                rden = pool.tile([128, P], F32, tag="rden")
                out5 = pool.tile([128, P], BF16, tag="out5")
                hr1 = pool.tile([128, P // 2], BF16, tag="hr1")
                outT = pool.tile([128, P // 2], F32, tag="outT")

                pse4 = [pp2.tile([128, SEGW], F32, tag="pse4",
                                 name=f"pse4_{s}") for s in range(4)]
                s4 = [pp2.tile([128, SEGW], F32, tag="mm128",
                               name=f"s4_{s}") for s in range(4)]
                psr1 = [pp2.tile([64, SEGW], F32, tag="psr1",
                                 name=f"psr1_{s}") for s in range(4)]
                psr2 = [pp2.tile([128, SEGW], F32, tag="pse4",
                                 name=f"psr2_{p}") for p in range(2)]

                for s in range(4):
                    cs = segs[s]
                    g, pr = 32 * (s % 2), prs[s // 2]
                    j = s % 2
                    nc.tensor.matmul(
                        pse4[s][:, :],
                        CR[:, C_PHI2 + 128 * j:C_PHI2 + 128 * (j + 1)],
                        h1[:, pr], start=True, stop=True)
                    nc.scalar.activation(enc4[:, cs], pse4[s][:, :], AF.Relu,
                                         bias=CP[:, S_BPHI4:S_BPHI4 + 1])
                    nc.tensor.matmul(s4[s][:, :],
                                     CR[0:64, C_WSA2 + 128 * j:
                                        C_WSA2 + 128 * (j + 1)],
                                     comb[0:64, pr],
                                     start=True, stop=False)
                    nc.tensor.matmul(s4[s][:, :],
                                     CR[:, C_WSF4 + 128 * s:
                                        C_WSF4 + 128 * (s + 1)],
                                     agg[:, :],
                                     start=False, stop=True)
                    # |logits| small for this model family: no max-shift
                    nc.scalar.activation(w4[:, cs], s4[s][:, :], AF.Exp)
                    xeng = nc.gpsimd if s < 3 else nc.vector
                    xeng.tensor_tensor(X[:, cs], w4[:, cs],
                                       enc4[:, cs], OP.mult)
                    # scans chained across segs: numl/denl are globally
                    # cumulative, so no carry application is needed at all
                    initn = 0.0 if s == 0 else numl[:, s * SEGW - 1:s * SEGW]
                    nc.vector.tensor_tensor_scan(
                        numl[:, cs], X[:, cs], X[:, cs], initn,
                        op0=OP.add, op1=OP.bypass)
                    initd = 0.0 if s == 0 else denl[:, s * SEGW - 1:s * SEGW]
                    nc.vector.tensor_tensor_scan(
                        denl[:, cs], w4[:, cs], w4[:, cs], initd,
                        op0=OP.add, op1=OP.bypass)
                    nc.vector.reciprocal_approx_fast(rden[:, cs],
                                                     denl[:, cs])
                    o5eng = nc.gpsimd if s < 3 else nc.vector
                    o5eng.tensor_tensor(out5[:, cs], numl[:, cs],
                                        rden[:, cs], OP.mult)
                if depth <= 4:
                    nc.sync.dma_start(out[:, :], w4[0:64, :])
                    return

                # ---------------- rho MLP ---------------------------------
                for s in range(4):
                    nc.tensor.matmul(psr1[s][:, :], CR[:, C_R1:C_R1 + 64],
                                     out5[:, segs[s]], start=True, stop=True)
                    nc.scalar.activation(
                        hr1[64 * (s % 2):64 * (s % 2 + 1), prs[s // 2]],
                        psr1[s][:, :], AF.Relu,
                        bias=CP[0:64, S_RB1X2:S_RB1X2 + 1])
                    if s % 2 == 0:
                        continue
                    # emit each pair's tail right after its second hr1 so the
                    # scheduler keeps the chain-critical ops ahead
                    p = s // 2
                    nc.tensor.matmul(psr2[p][:, :], CR[:, C_R2B:C_R2B + 128],
                                     hr1[:, prs[p]], start=True, stop=True)
                    nc.scalar.activation(outT[:, prs[p]], psr2[p][:, :],
                                         AF.Relu,
                                         bias=CP[:, S_RB2X2:S_RB2X2 + 1])
                    for j in range(2):
                        eng = nc.sync if j == 0 else nc.scalar
                        eng.dma_start(
                            out[:, segs[2 * p + j]],
                            outT[64 * j:64 * (j + 1), prs[p]])

            if repeat == 1:
                body()
            else:
                with tc.For_i(0, repeat, 1):
                    body()

    nc.compile()
    return nc


def host_prep(inputs):
    """Fold parameters on the host; returns the packed const tensor."""
    f = lambda k: np.ascontiguousarray(np.asarray(inputs[k], np.float32))
    W_k, W_q = f("W_k"), f("W_q")
    Wq_exp = np.zeros((DOT * HEADS, HEADS), np.float32)
    for h in range(HEADS):
        for d in range(DOT):
            Wq_exp[d * HEADS + h, h] = W_q[h, d]
    Wpre = (W_k @ Wq_exp) / np.sqrt(np.float32(DOT))   # [63, 4]
    wpre_a = Wpre[:DIM_S]
    wfold = f("arho_w") @ Wpre[DIM_S:]                  # [32, 4]
    rep = np.repeat(np.arange(HEADS), PHI_W)            # [128]
    # comb row permutation: group order [pad, onehot22, values, tenc8]
    perm = np.concatenate([np.arange(9, 31), [8], np.arange(0, 8)])
    pad1 = lambda a: np.vstack([np.zeros((1, a.shape[1]), np.float32), a])
    wsa = pad1(np.ascontiguousarray(wpre_a[perm][:, rep]))   # [32, 128]
    wsf = np.ascontiguousarray(wfold[:, rep])           # [32, 128]
    w1p = pad1(np.ascontiguousarray(
        np.hstack([f("psi_w1"), f("phi_w1")])[perm]))   # [32, 64]

    psi2 = f("psi_w2")                                  # [32, 32]
    phi2rep = np.vstack([np.zeros((32, 128), np.float32),
                         np.tile(f("phi_w2"), (1, HEADS))])  # [64, 128]

    posvec = np.power(10000.0, 2.0 * (np.arange(D_TIME) // 2) / D_TIME)
    scale2pi = (1.0 / (posvec * 2 * np.pi)).astype(np.float32)
    shift2pi = np.where(np.arange(D_TIME) % 2 == 0, 0.0, 0.25).astype(
        np.float32)

    cp = np.zeros((128, NCW), np.float32)
    # W1X2: blockdiag over the comb pair groups -> h1 [even64 | odd64]
    cp[0:32, C_W1X2:C_W1X2 + 64] = w1p
    cp[32:64, C_W1X2 + 64:C_W1X2 + 128] = w1p
    # psi2 pieces: h1 rows 0..31 = psi-hidden even seg, 64..95 = odd seg;
    # pspsi rows 32s = seg s (s order 0,1 from pair0; 2,3 from pair1)
    cp[0:32, C_W2PL + 0:C_W2PL + 32] = psi2
    cp[64:96, C_W2PL + 32:C_W2PL + 64] = psi2
    cp[0:32, C_W2PH + 64:C_W2PH + 96] = psi2
    cp[64:96, C_W2PH + 96:C_W2PH + 128] = psi2
    # phi2 per parity: contract only the matching 64-row half of h1
    cp[0:64, C_PHI2:C_PHI2 + 128] = phi2rep
    cp[64:128, C_PHI2 + 128:C_PHI2 + 256] = phi2rep
    # wsa per parity: contract only the matching comb row group
    cp[0:32, C_WSA2:C_WSA2 + 128] = wsa
    cp[32:64, C_WSA2 + 128:C_WSA2 + 256] = wsa
    # wsf per seg: contract only agg partition group s
    for s in range(4):
        cp[32 * s:32 * (s + 1), C_WSF4 + 128 * s:C_WSF4 + 128 * (s + 1)] = wsf
    cp[:, C_R1:C_R1 + 64] = f("rho_w1")
    cp[0:64, C_R2B:C_R2B + 64] = f("rho_w2")
    cp[64:128, C_R2B + 64:C_R2B + 128] = f("rho_w2")
    t = np.zeros((128, 128), np.float32)
    for s in range(1, 4):
        for sp in range(s):
            t[np.arange(32) + 32 * sp, np.arange(32) + 32 * s] = 1.0
    cp[:, C_T128:C_T128 + 128] = t

    b1cat = np.concatenate([f("psi_b1"), f("phi_b1")])
    cp[:, CW_END + S_B1X2] = np.tile(b1cat, 2)
    cp[:, CW_END + S_BPSI4] = np.tile(f("psi_b2"), 4)
    cp[:, CW_END + S_BPHI4] = np.tile(f("phi_b2"), HEADS)
    cp[:, CW_END + S_RB1X2] = np.tile(f("rho_b1"), 2)
    cp[:, CW_END + S_RB2X2] = np.tile(f("rho_b2"), 2)
    cp[:, CW_END + S_SC] = np.repeat(scale2pi, NCH)
    cp[:, CW_END + S_SH] = np.repeat(shift2pi, NCH)
    # io: iota 1..22 in rows 1..22 / 33..54, -1 in the pad rows so the
    # one-hot compare zero-fills them
    cp[:, CW_END + S_IO] = -1.0
    io = np.arange(1, NUM_MODS + 1, dtype=np.float32)
    cp[1:1 + NUM_MODS, CW_END + S_IO] = io
    cp[33:33 + NUM_MODS, CW_END + S_IO] = io
    pos = np.arange(P, dtype=np.float32).reshape(4, SEGW)
    cp[:, CW_END + S_RC:CW_END + S_RC + SEGW] = np.repeat(
        1.0 / (pos + 1.0), 32, axis=0)
    import ml_dtypes
    cpw = np.ascontiguousarray(cp[:, 0:CW_END]).astype(ml_dtypes.bfloat16)
    cps = np.ascontiguousarray(cp[:, CW_END:NCW])
    return cpw, cps


def make_in_maps(inputs):
    cpw, cps = host_prep(inputs)
    times = np.asarray(inputs["times"], np.float32)
    values = np.asarray(inputs["values"], np.float32)
    meas = np.asarray(inputs["measurements"]).astype(np.float32)
    in_maps = []
    for b in range(B):
        in_maps.append({
            "cpw": cpw,
            "cps": cps,
            "times": np.ascontiguousarray(times[b][None, :]),
            "values": np.ascontiguousarray(values[b][None, :]),
            "meas": np.ascontiguousarray(meas[b][None, :]),
        })
    return in_maps


_NC_CACHE = {}


def _get_nc(repeat=1):
    if repeat not in _NC_CACHE:
        _NC_CACHE[repeat] = build(repeat)
    return _NC_CACHE[repeat]


def kernel(**inputs) -> np.ndarray:
    nc = _get_nc(1)
    in_maps = make_in_maps(inputs)
    res = bass_utils.run_bass_kernel_spmd(
        nc, in_maps, core_ids=list(range(N_CORES)))
    outs = [np.ascontiguousarray(res.results[b]["out"].T) for b in range(B)]
    return np.stack(outs, 0).astype(np.float32)
